# revision 22
# baseline (speedup 1.0000x reference)
"""GRU-ODE delay cell on 8 Trainium2 NeuronCores (Bass/Tile), fp8 DoubleRow.

Math (per reference):
    x   = x_coeffs[int(t)]                  # [B, I]
    r   = sigmoid([x, h] @ W_r.T)
    z   = sigmoid([x, h] @ W_z.T)
    h~  = tanh([x, r*h] @ W_h.T)
    dh  = (1 - z) * (h~ - h)

Data-parallel over batch (B=8192 -> 1024 rows/core), weights replicated,
transposed ([feature, batch]) layout throughout.

Precision scheme (max rel err ~1.5e-2 vs 2e-2 budget, sim-verified):
  - r gate and candidate gate h-parts run as fp8e4 DoubleRow matmuls:
    one DR pass contracts a 256-deep k-pair in the same PE time as a
    128-deep bf16 matmul (2x FLOPs). Per 512-wide psum half the group is
    [x-part bf16 (start), 4 DR passes (stop)] = 5 mms instead of 9.
  - z gate stays bf16 for its first NBF_Z h-tiles (its error multiplies
    the large (h~-h) term); remaining tiles are fp8 DR pairs.
  - all weights are pre-scaled by 32 on the host so fp8 weights stay out
    of the e4m3 subnormal range; activations fold in scale=+-1/32.
  - zm/d intermediates fp32, output fp16, rh quantized to fp8 on DVE.
"""

import numpy as np
import ml_dtypes

B, H, I, TMAX = 8192, 1024, 128, 128
NCORES = 8
BC = B // NCORES          # batch rows per core
NT = H // 128             # 8 hidden output tiles
NP = 4                    # h k-tile pairs (DoubleRow contracts 2 tiles/pass)
MM_N = 512                # moving free-dim per matmul (one PSUM bank of fp32)
NBF_Z = 4                 # z-gate h-tiles computed in bf16 (rest: fp8 DR)
WS = 32.0                 # host-side weight scale (fp8 subnormal avoidance)

_BF16 = ml_dtypes.bfloat16
_FP8 = ml_dtypes.float8_e4m3

_cache = {}


def _build_nc():
    import concourse.bacc as bacc
    import concourse.tile as tile
    import concourse.mybir as mybir

    f32 = mybir.dt.float32
    f16 = mybir.dt.float16
    bf16 = mybir.dt.bfloat16
    fp8 = mybir.dt.float8e4
    AF = mybir.ActivationFunctionType
    DR = mybir.MatmulPerfMode.DoubleRow
    NPZ = (NT - NBF_Z + 1) // 2      # z-gate fp8 pairs

    nc = bacc.Bacc(
        "TRN2",
        target_bir_lowering=False,
        debug=False,
        enable_asserts=False,
        num_devices=NCORES,
    )

    # DRAM layouts mirror SBUF tile shapes (host pre-packs).
    # x and the r-gate x-part weights ship as ONE stream: they are the
    # first-needed bytes and a single descriptor lands them sooner.
    xw_d = nc.dram_tensor("xw", [128, BC + H], bf16, kind="ExternalInput").ap()
    hTb_d = nc.dram_tensor("hTb", [128, NT, BC], bf16, kind="ExternalInput").ap()
    hT8_d = nc.dram_tensor("hT8", [NP, 128, 2, BC], fp8, kind="ExternalInput").ap()
    wzx_d = nc.dram_tensor("wzx", [128, H], bf16, kind="ExternalInput").ap()
    whx_d = nc.dram_tensor("whx", [128, H], bf16, kind="ExternalInput").ap()
    wr8_d = nc.dram_tensor("wr8", [NP, 128, 2, H], fp8, kind="ExternalInput").ap()
    wh8_d = nc.dram_tensor("wh8", [NP, 128, 2, H], fp8, kind="ExternalInput").ap()
    if NBF_Z:
        wzb_d = nc.dram_tensor("wzb", [128, NBF_Z, H], bf16, kind="ExternalInput").ap()
    if NPZ:
        wz8_d = nc.dram_tensor("wz8", [NPZ, 128, 2, H], fp8, kind="ExternalInput").ap()
    dh_d = nc.dram_tensor("dhT", [NT, 128, BC], f16, kind="ExternalOutput").ap()
    # sink for the PE warm-up matmuls (keeps them from being DCE'd)
    warm_d = nc.dram_tensor("warm", [128, 4], f32, kind="ExternalOutput").ap()

    bhalves = [(j * MM_N, MM_N) for j in range(BC // MM_N)]

    with tile.TileContext(nc) as tc:
        with (
            tc.tile_pool(name="res", bufs=1) as res,
            tc.tile_pool(name="work", bufs=3) as work,
            tc.tile_pool(name="psum", bufs=4, space="PSUM") as psum,
        ):
            # ---- PE warm-up: keep the PE busy through the DMA-latency
            # window so the clock is ramped when the first real mm issues.
            warm_in = res.tile([128, 512], bf16, name="warm_in", tag="warm_in")
            nc.vector.memset(warm_in[:], 0.0)
            warm_ps = psum.tile([128, 512], f32, name="warm_ps", tag="ps")
            for _ in range(3):
                nc.tensor.matmul(
                    warm_ps[:], warm_in[:, :128], warm_in[:], start=True, stop=True
                )
            warm_sb = res.tile([128, 4], f32, name="warm_sb", tag="warm_sb")
            nc.vector.tensor_copy(warm_sb[:], warm_ps[:, :4])
            nc.gpsimd.dma_start(warm_d[:], warm_sb[:])

            # ---- resident loads ----
            # ALL loads go on sync, ordered by first-use time. Issuing from
            # several engines at t=0 makes the DMA rings round-robin all 8MB
            # and the r-gate prefix lands 20us late; serial issue on sync
            # (~0.6-1.2us descriptor gen each) staggers the streams so the
            # critical 2.5MB r-gate prefix gets the full bandwidth first.
            xw_sb = res.tile([128, BC + H], bf16, name="xw_sb", tag="xw_sb")
            nc.sync.dma_start(xw_sb[:], xw_d[:])

            def x_ap(b0, bw):
                return xw_sb[:, b0:b0 + bw]

            def wrx_ap(n):
                return xw_sb[:, BC + n * 128:BC + (n + 1) * 128]
            wr8_sb, h8_sb = [], []
            for p in range(NP):
                t = res.tile([128, 2, H], fp8, name=f"wr8_{p}", tag=f"wr8_{p}")
                nc.sync.dma_start(t[:], wr8_d[p])
                wr8_sb.append(t)
                t = res.tile([128, 2, BC], fp8, name=f"h8_{p}", tag=f"h8_{p}")
                nc.sync.dma_start(t[:], hT8_d[p])
                h8_sb.append(t)

            # h (bf16): first rh mul needs tile 0 shortly after sigmoid r0
            hb_sb = res.tile([128, NT, BC], bf16, name="hb_sb", tag="hb")
            nc.sync.dma_start(hb_sb[:], hTb_d[:])

            # candidate-gate weights (needed ~halfway in). Keeping a single
            # DMA issuer (sync) preserves stream ordering at the ring level;
            # a second issuer steals bandwidth from the r-gate prefix.
            whx_sb = res.tile([128, H], bf16, name="whx_sb", tag="whx")
            nc.sync.dma_start(whx_sb[:], whx_d[:])
            wh8_sb = []
            for p in range(NP):
                t = res.tile([128, 2, H], fp8, name=f"wh8_{p}", tag=f"wh8_{p}")
                nc.sync.dma_start(t[:], wh8_d[p])
                wh8_sb.append(t)
            wzx_sb = res.tile([128, H], bf16, name="wzx_sb", tag="wzx")
            nc.sync.dma_start(wzx_sb[:], wzx_d[:])
            if NBF_Z:
                wzb_sb = res.tile([128, NBF_Z, H], bf16, name="wzb_sb", tag="wzb")
                nc.sync.dma_start(wzb_sb[:], wzb_d[:])
            wz8_sb = []
            for p in range(NPZ):
                t = res.tile([128, 2, H], fp8, name=f"wz8_{p}", tag=f"wz8_{p}")
                nc.sync.dma_start(t[:], wz8_d[p])
                wz8_sb.append(t)

            rh8_sb = [
                res.tile([128, 2, BC], fp8, name=f"rh8_{p}", tag=f"rh8_{p}")
                for p in range(NP)
            ]
            # zm = 1 - z persists for the two z tiles computed early
            zm_sb = [
                res.tile([128, BC], f32, name=f"zm{k}", tag=f"zm{k}")
                for k in range(2)
            ]
            # d = (h~ - h) persists for tiles whose z gate runs last
            d_sb = [
                res.tile([128, BC], f32, name=f"d{k}", tag=f"d{k}")
                for k in range(2, NT)
            ]

            def hb(n):
                return hb_sb[:, n, :]

            def rw8(p, n):
                return wr8_sb[p][:, :, n * 128:(n + 1) * 128]

            def rmov(p, b0, bw):
                return h8_sb[p][:, :, b0:b0 + bw]

            def hw8(p, n):
                return wh8_sb[p][:, :, n * 128:(n + 1) * 128]

            def hmov(p, b0, bw):
                return rh8_sb[p][:, :, b0:b0 + bw]

            def xmm(ps, n, wxf, b0, bw):
                nc.tensor.matmul(
                    ps[:, b0:b0 + bw], wxf(n), x_ap(b0, bw),
                    start=True, stop=False,
                )

            def dr_mms(ps, n, wsl, msl, b0, bw):
                sl = slice(b0, b0 + bw)
                for p in range(NP):
                    nc.tensor.matmul(
                        ps[:, sl], wsl(p, n), msl(p, b0, bw),
                        start=False, stop=(p == NP - 1), perf_mode=DR,
                    )

            def fp8_gate_mms(ps, n, wxf, wsl, msl, b0, bw):
                """[x-part bf16 (start), NP DoubleRow fp8 (stop)] group."""
                xmm(ps, n, wxf, b0, bw)
                dr_mms(ps, n, wsl, msl, b0, bw)

            def z_gate_mms(ps, n, b0, bw):
                """[x-part, NBF_Z bf16 h-tiles, NPZ DR pairs] group."""
                sl = slice(b0, b0 + bw)
                cols = slice(n * 128, (n + 1) * 128)
                nc.tensor.matmul(
                    ps[:, sl], wzx_sb[:, cols], x_ap(b0, bw),
                    start=True, stop=False,
                )
                for k in range(NBF_Z):
                    nc.tensor.matmul(
                        ps[:, sl], wzb_sb[:, k, cols], hb(k)[:, sl],
                        start=False, stop=(k == NT - 1),
                    )
                for p in range(NPZ):
                    pp = NBF_Z // 2 + p
                    nc.tensor.matmul(
                        ps[:, sl], wz8_sb[p][:, :, cols], rmov(pp, b0, bw),
                        start=False, stop=(p == NPZ - 1), perf_mode=DR,
                    )

            # ---- r gate ----
            # The x-part matmuls of the first 4 tiles are hoisted to the
            # front: they only need x+wrx (~0.5MB, first loads) and act as
            # PE warm-up that is real work while the fp8 prefix streams in.
            HOIST = 4
            ps_r = []
            for n in range(HOIST):
                ps = psum.tile([128, BC], f32, name="ps_r", tag="ps")
                ps_r.append(ps)
                for b0, bw in bhalves:
                    xmm(ps, n, wrx_ap, b0, bw)
            for n in range(NT):
                if n < HOIST:
                    ps = ps_r[n]
                    for b0, bw in bhalves:
                        dr_mms(ps, n, rw8, rmov, b0, bw)
                else:
                    ps = psum.tile([128, BC], f32, name="ps_r", tag="ps")
                    for b0, bw in bhalves:
                        fp8_gate_mms(ps, n, wrx_ap, rw8, rmov, b0, bw)
                r_t = work.tile([128, BC], f32, name="r_t", tag="r_t")
                nc.scalar.activation(r_t[:], ps[:], AF.Sigmoid, scale=1.0 / WS)
                # rh = fp8(r * h) into the DR pair layout
                nc.vector.tensor_mul(rh8_sb[n // 2][:, n % 2, :], r_t[:], hb(n))

            # ---- z gate, first two tiles (zm = 1 - z = sigmoid(-pre)) ----
            for n in range(2):
                ps = psum.tile([128, BC], f32, name="ps_z", tag="ps")
                for b0, bw in bhalves:
                    z_gate_mms(ps, n, b0, bw)
                nc.scalar.activation(zm_sb[n][:], ps[:], AF.Sigmoid, scale=-1.0 / WS)

            # ---- candidate gate ----
            for n in range(NT):
                ps = psum.tile([128, BC], f32, name="ps_h", tag="ps")
                for b0, bw in bhalves:
                    fp8_gate_mms(ps, n, lambda nn: whx_sb[:, nn * 128:(nn + 1) * 128], hw8, hmov, b0, bw)
                for b0, bw in bhalves:
                    sl = slice(b0, b0 + bw)
                    ht = work.tile([128, bw], f32, name="ht", tag="ht")
                    nc.scalar.activation(ht[:], ps[:, sl], AF.Tanh, scale=1.0 / WS)
                    if n < 2:
                        # z already known: finish dh = zm * (h~ - h) now
                        d_t = work.tile([128, bw], f32, name="d_t", tag="d_t")
                        nc.vector.tensor_sub(d_t[:], ht[:], hb(n)[:, sl])
                        o_t = work.tile([128, bw], f16, name="o_t", tag="o_t")
                        nc.vector.tensor_mul(o_t[:], d_t[:], zm_sb[n][:, sl])
                        nc.sync.dma_start(dh_d[n][:, sl], o_t[:])
                    else:
                        # stash h~ - h; z for this tile is computed afterwards
                        nc.vector.tensor_sub(d_sb[n - 2][:, sl], ht[:], hb(n)[:, sl])

            # ---- z gate, remaining tiles + output ----
            for n in range(2, NT):
                ps = psum.tile([128, BC], f32, name="ps_z2", tag="ps")
                for b0, bw in bhalves:
                    z_gate_mms(ps, n, b0, bw)
                # the very last half is processed in 256-col quarters on
                # alternating DMA issuers to shorten the post-matmul tail
                chunks = [(b0, bw) for b0, bw in bhalves]
                if n == NT - 1:
                    b0, bw = chunks.pop()
                    chunks += [(b0, 256), (b0 + 256, 128), (b0 + 384, 128)]
                for i, (b0, bw) in enumerate(chunks):
                    sl = slice(b0, b0 + bw)
                    zm_t = work.tile([128, bw], f32, name="zm_t", tag="zm_t")
                    nc.scalar.activation(zm_t[:], ps[:, sl], AF.Sigmoid, scale=-1.0 / WS)
                    o_t = work.tile([128, bw], f16, name="o_t", tag="o_t")
                    nc.vector.tensor_mul(o_t[:], zm_t[:], d_sb[n - 2][:, sl])
                    if n == NT - 1 and i == len(chunks) - 1:
                        nc.scalar.dma_start(dh_d[n][:, sl], o_t[:])
                    else:
                        nc.sync.dma_start(dh_d[n][:, sl], o_t[:])

    nc.compile()
    return nc


def _pack_weights(W_r, W_z, W_h):
    def xpart(W):
        return np.ascontiguousarray((W[:, :I] * WS).T).astype(_BF16)  # [128, H]

    def pairs(W):
        w = np.ascontiguousarray((W[:, I:] * WS).T).astype(_FP8)      # [H, H]
        return np.ascontiguousarray(w.reshape(NP, 2, 128, H).transpose(0, 2, 1, 3))

    wrx, wzx, whx = xpart(W_r), xpart(W_z), xpart(W_h)
    wr8 = pairs(W_r)
    wh8 = pairs(W_h)
    out = {"wzx": wzx, "whx": whx, "wr8": wr8, "wh8": wh8}
    out["_wrx"] = wrx
    if NBF_Z:
        wzb = np.ascontiguousarray((W_z[:, I:I + NBF_Z * 128] * WS).T).astype(_BF16)
        out["wzb"] = np.ascontiguousarray(
            wzb.reshape(NBF_Z, 128, H).transpose(1, 0, 2))   # [128, NBF_Z, H]
    NPZ = (NT - NBF_Z + 1) // 2
    if NPZ:
        wz8f = np.ascontiguousarray((W_z[:, I + NBF_Z * 128:] * WS).T).astype(_FP8)
        out["wz8"] = np.ascontiguousarray(
            wz8f.reshape(NPZ, 2, 128, H).transpose(0, 2, 1, 3))
    return out


def _prep_core_inputs(x, h, wpack):
    maps = []
    wrx = wpack.pop("_wrx")
    for c in range(NCORES):
        s = slice(c * BC, (c + 1) * BC)
        xT = x[s].T.astype(_BF16)                                    # [128, BC]
        xw = np.ascontiguousarray(np.concatenate([xT, wrx], axis=1))
        hT = np.ascontiguousarray(h[s].T)                            # [H, BC]
        hTb = np.ascontiguousarray(
            hT.astype(_BF16).reshape(NT, 128, BC).transpose(1, 0, 2))  # [128,NT,BC]
        hT8 = np.ascontiguousarray(
            hT.astype(_FP8).reshape(NP, 2, 128, BC).transpose(0, 2, 1, 3))
        m = {"xw": xw, "hTb": hTb, "hT8": hT8}
        m.update(wpack)
        maps.append(m)
    return maps


def _ensure_axon_hooks_importable():
    """bass_utils imports antenv.axon_hooks when tracing is requested; some
    images ship an antenv stub without it. Provide a no-op fallback so a
    stray BASS_TRACE env var can't crash the run."""
    import sys

    try:
        import antenv.axon_hooks  # noqa: F401
    except ImportError:
        import types

        mod = types.ModuleType("antenv.axon_hooks")
        mod.get_axon_ntff_profile_hook = lambda: None
        mod.set_axon_ntff_profile_hook = lambda h: None
        sys.modules["antenv.axon_hooks"] = mod


def kernel(t, h, x_coeffs, W_r, W_z, W_h):
    _ensure_axon_hooks_importable()
    from concourse.bass_utils import run_bass_kernel_spmd

    t = np.asarray(t)
    h = np.asarray(h, dtype=np.float32)
    x_coeffs = np.asarray(x_coeffs)
    W_r = np.asarray(W_r, dtype=np.float32)
    W_z = np.asarray(W_z, dtype=np.float32)
    W_h = np.asarray(W_h, dtype=np.float32)

    t_int = int(np.clip(np.int32(float(t)), 0, x_coeffs.shape[0] - 1))
    x = np.asarray(x_coeffs[t_int], dtype=np.float32)                # [B, I]

    if "nc" not in _cache:
        _cache["nc"] = _build_nc()
    nc = _cache["nc"]

    wpack = _pack_weights(W_r, W_z, W_h)
    in_maps = _prep_core_inputs(x, h, wpack)

    import os

    trace = bool(os.environ.get("BASS_TRACE"))
    res = run_bass_kernel_spmd(nc, in_maps, list(range(NCORES)), trace=trace)
    _cache["last_result"] = res

    outs = []
    for c in range(NCORES):
        dhT = res.results[c]["dhT"]                                  # [NT,128,BC]
        outs.append(dhT.reshape(H, BC))
    dhT_full = np.concatenate(outs, axis=1)                          # [H, B]
    return np.ascontiguousarray(dhT_full.T).astype(np.float32)       # [B, H]
